# revision 1
# baseline (speedup 1.0000x reference)
"""Modulated Conv2D (StyleGAN2-style) Trainium2 Bass kernel.

Problem shapes (hardcoded):
  x: [16, 256, 64, 64] f32    y: [16, 512] f32
  weights: [256, 256, 3, 3]   bias: [256]
  style_w: [256, 512]         style_b: [256]
  out: [16, 256, 64, 64] f32

Math identity used: instead of materializing per-sample modulated weights,
  out[b,o] = (1/wstd[b,o]) * conv(x[b] * style[b,:], w)[o] + bias[o]
  wstd[b,o] = sqrt(sum_i W2[o,i] * style[b,i]^2 + eps),  W2[o,i] = sum_kk w[o,i,kk]^2
so the conv weights are batch-independent (shared across samples/cores).

Sharding: data-parallel over batch, 2 samples per core across 8 cores.
Conv computed as 9 shifted matmuls (per 3x3 tap) accumulating in PSUM,
bf16 operands with f32 accumulation.

Engine assignment: PE transposes weights (f32) + conv matmuls; ScalarE does
the transpose-PSUM drains (f32->bf16) and the output scale+bias; DVE does
style and the x scale-casts; GPSIMD does W2 = sum w^2 and pad memsets.
"""

import numpy as np

import concourse.bass as bass
import concourse.tile as tile
from concourse import bacc, mybir
from concourse import bass_utils
from concourse.masks import make_identity

EPS = 1e-8
P = 128
B_LOC = 2          # samples per core
CIN, COUT = 256, 256
NI, NO = CIN // P, COUT // P   # 2, 2
S = 512
H = W = 64
KK = 9             # 3x3 taps
HP, WP = H + 2, W + 2  # zero-padded image
N_CORES = 8
ROWS_A = 34        # first-chunk rows of the x load/cast split (covers half0 reads)

F32 = mybir.dt.float32
BF16 = mybir.dt.bfloat16
AF = mybir.ActivationFunctionType

# HW-bisection flags (all True = fastest per TimelineSim; flipped off to
# isolate hardware-only failures)
import os as _os
GROUPED_T = _os.environ.get("K_GROUPED_T", "0") == "1"   # multi-transpose per PSUM bank
MID_T01 = _os.environ.get("K_MID_T01", "0") == "1"       # T01 inside first mm block
USE_TTR = _os.environ.get("K_USE_TTR", "0") == "1"       # TensorTensorReduce for style
VEC_OUT = _os.environ.get("K_VEC_OUT", "0") == "1"       # DVE drain for last blocks
FINE_TAIL = _os.environ.get("K_FINE_TAIL", "0") == "1"   # eighth-sized tail blocks


def build_conv2dmod(nc):
    x = nc.dram_tensor("x", [B_LOC, CIN, H, W], F32, kind="ExternalInput")
    y = nc.dram_tensor("y", [B_LOC, S], F32, kind="ExternalInput")
    weights = nc.dram_tensor("weights", [COUT, CIN, 3, 3], F32, kind="ExternalInput")
    bias = nc.dram_tensor("bias", [COUT], F32, kind="ExternalInput")
    style_w = nc.dram_tensor("style_w", [CIN, S], F32, kind="ExternalInput")
    style_b = nc.dram_tensor("style_b", [CIN], F32, kind="ExternalInput")
    out = nc.dram_tensor("out", [B_LOC, COUT, H, W], F32, kind="ExternalOutput")

    with tile.TileContext(nc) as tc:
        with (
            tc.tile_pool(name="consts", bufs=1) as consts,
            tc.tile_pool(name="temps", bufs=2) as temps,
            tc.tile_pool(name="xin_pool", bufs=1) as xin_pool,
            tc.tile_pool(name="xs_pool", bufs=1) as xs_pool,
            tc.tile_pool(name="out_pool", bufs=3) as out_pool,
            tc.tile_pool(name="psum", bufs=2, space="PSUM") as psum,
        ):
            # ---------- DMA loads, split + ordered by when they gate compute -----
            sw_nat = [consts.tile([P, S], F32, name=f"sw_nat{i}", tag=f"sw_nat{i}")
                      for i in range(NI)]
            y_bcast = consts.tile([P, B_LOC, S], F32)
            w_nat = [consts.tile([P, CIN, KK], F32, name=f"w_nat{o}", tag=f"w_nat{o}")
                     for o in range(NO)]
            xin = {}
            for b in range(B_LOC):
                for it in range(NI):
                    xin[(b, it)] = xin_pool.tile([P, H, W], F32,
                                                 name=f"xin{b}_{it}", tag=f"xin{b}_{it}")

            def load_xin(b, it, part):
                r = slice(0, ROWS_A) if part == 0 else slice(ROWS_A, H)
                nc.sync.dma_start(xin[(b, it)][:, r, :],
                                  x.ap()[b, it * P:(it + 1) * P, r, :])

            def load_w(ot, ih):
                nc.sync.dma_start(
                    w_nat[ot][:, ih * P:(ih + 1) * P, :],
                    weights.ap()[ot * P:(ot + 1) * P, ih * P:(ih + 1) * P]
                    .rearrange("o i kh kw -> o i (kh kw)"),
                )

            load_w(0, 0)
            nc.sync.dma_start(y_bcast[:], y.ap()[None].to_broadcast((P, B_LOC, S)))
            nc.sync.dma_start(sw_nat[0][:], style_w.ap()[0:P, :])
            bias_col = consts.tile([P, NO], F32)
            nc.sync.dma_start(bias_col[:], bias.ap().rearrange("(oo oi) -> oi oo", oi=P))
            style_b_col = consts.tile([P, NI], F32)
            nc.sync.dma_start(style_b_col[:], style_b.ap().rearrange("(io ii) -> ii io", ii=P))
            load_xin(0, 0, 0)
            nc.sync.dma_start(sw_nat[1][:], style_w.ap()[P:2 * P, :])
            load_w(0, 1)
            load_xin(0, 1, 0)
            load_xin(0, 0, 1)
            load_xin(0, 1, 1)
            load_w(1, 0)
            load_w(1, 1)
            for it in range(NI):
                for part in range(2):
                    load_xin(1, it, part)

            # ---------- GPSIMD: identity, y broadcast, b0 pad borders ----------
            identity_bf = consts.tile([P, P], BF16)
            make_identity(nc, identity_bf)

            # pre-warm the ACT function table that Sqrt lives in, so the
            # LoadActFuncSet doesn't fire mid-kernel on the demod path
            eps_col = consts.tile([P, 1], F32)
            nc.gpsimd.memset(eps_col[:], EPS)
            lafs_warm = consts.tile([P, 1], F32)
            nc.scalar.activation(lafs_warm[:], eps_col[:], AF.Sqrt)

            xs = {}

            def xs_borders(b):
                for it in range(NI):
                    xp = xs_pool.tile([P, HP, WP], BF16, name=f"xs{b}_{it}",
                                      tag=f"xs{b}_{it}")
                    nc.gpsimd.memset(xp[:, 0, :], 0.0)
                    nc.gpsimd.memset(xp[:, HP - 1, :], 0.0)
                    nc.gpsimd.memset(xp[:, 1:HP - 1, 0], 0.0)
                    nc.gpsimd.memset(xp[:, 1:HP - 1, WP - 1], 0.0)
                    xs[(b, it)] = xp

            xs_borders(0)

            # ---------- style (DVE): fused multiply-reduce per (it, b) ----------
            style_col = []
            style2 = []
            for it in range(NI):
                sc = consts.tile([P, B_LOC], F32, name=f"style_col{it}", tag=f"style_col{it}")
                s2 = consts.tile([P, B_LOC], F32, name=f"style2{it}", tag=f"style2{it}")
                if USE_TTR:
                    for b in range(B_LOC):
                        junk = temps.tile([P, S], F32, name=f"junk{it}_{b}", tag="junk")
                        nc.vector.tensor_tensor_reduce(
                            out=junk[:], in0=sw_nat[it][:], in1=y_bcast[:, b, :],
                            scale=1.0, scalar=0.0,
                            op0=mybir.AluOpType.mult, op1=mybir.AluOpType.add,
                            accum_out=sc[:, b:b + 1],
                        )
                        nc.vector.tensor_scalar_add(sc[:, b:b + 1], sc[:, b:b + 1],
                                                    style_b_col[:, it:it + 1])
                        nc.vector.tensor_mul(s2[:, b:b + 1], sc[:, b:b + 1],
                                             sc[:, b:b + 1])
                else:
                    tmp = temps.tile([P, B_LOC, S], F32, name=f"stmp{it}", tag="junk")
                    nc.vector.tensor_mul(
                        tmp[:], y_bcast[:],
                        sw_nat[it][:, None, :].to_broadcast((P, B_LOC, S)))
                    nc.vector.reduce_sum(sc[:], tmp[:], axis=mybir.AxisListType.X)
                    nc.vector.tensor_scalar_add(sc[:], sc[:], style_b_col[:, it:it + 1])
                    nc.vector.tensor_mul(s2[:], sc[:], sc[:])
                style_col.append(sc)
                style2.append(s2)

            # ---------- weights: bf16 cast (GPSIMD) + PE transpose + ACT drain ----
            w_nat_bf = [consts.tile([P, CIN, KK], BF16, name=f"w_nbf{o}", tag=f"w_nbf{o}")
                        for o in range(NO)]
            # kk-major so the PSUM drain and the matmul lhsT reads are contiguous
            w_bf = [consts.tile([P, KK, COUT], BF16, name=f"w_bf{i}", tag=f"w_bf{i}")
                    for i in range(NI)]
            tp_idx = [0]

            def cast_w(ot, ih, eng=None):
                (eng or nc.gpsimd).tensor_copy(
                    w_nat_bf[ot][:, ih * P:(ih + 1) * P, :],
                    w_nat[ot][:, ih * P:(ih + 1) * P, :])

            def transpose_wblock(ot, it):
                # w_nat_bf[ot][:, it_block, kk] --PE--> psum[i, o] --ACT--> w_bf
                if GROUPED_T:
                    # 4 transposes share one PSUM bank, drain in a single ACT copy
                    groups = ((0, 4), (4, 4), (8, 1))
                else:
                    groups = tuple((kk, 1) for kk in range(KK))
                for kk0, n in groups:
                    pt = psum.tile([P, n, P], BF16, name=f"tp{ot}_{it}_{kk0}",
                                   tag=f"ch{tp_idx[0] % 4}")
                    tp_idx[0] += 1
                    for j in range(n):
                        nc.tensor.transpose(
                            pt[:, j, :],
                            w_nat_bf[ot][:, it * P:(it + 1) * P, kk0 + j],
                            identity_bf[:],
                        )
                    nc.scalar.copy(
                        w_bf[it][:, kk0:kk0 + n, ot * P:(ot + 1) * P], pt[:]
                    )

            cast_w(0, 0, nc.vector)
            transpose_wblock(0, 0)

            # ---------- x scale+cast (DVE), ordered by need ----------
            def xs_cast(b, it, part):
                r = slice(0, ROWS_A) if part == 0 else slice(ROWS_A, H)
                return nc.vector.tensor_scalar_mul(
                    xs[(b, it)][:, r.start + 1:r.stop + 1, 1:W + 1],
                    xin[(b, it)][:, r, :],
                    style_col[it][:, b:b + 1],
                )

            xs_cast(0, 0, 0)
            xs_cast(0, 1, 0)
            xs_cast(0, 0, 1)
            last_cast_b0 = xs_cast(0, 1, 1)

            # ---------- main conv block: 18*nchunks matmuls per call ----------
            def mm_block(b, ot, r0, nchunks, ctag0=0, mid_cb=None):
                pcs = [psum.tile([P, 8, W], F32, name=f"pc{b}{ot}{r0}_{c}",
                                 tag=f"ch{(ctag0 + c) % 4}")
                       for c in range(nchunks)]
                first, last = (0, 0), (NI - 1, KK - 1)
                for it in range(NI):
                    if it == 1 and mid_cb is not None:
                        mid_cb()
                    for kk in range(KK):
                        dy, dx = kk // 3, kk % 3
                        lhsT = w_bf[it][:, kk, ot * P:(ot + 1) * P]
                        for c in range(nchunks):
                            rs = r0 + c * 8 + dy
                            nc.tensor.matmul(
                                pcs[c][:], lhsT, xs[(b, it)][:, rs:rs + 8, dx:dx + W],
                                start=((it, kk) == first), stop=((it, kk) == last),
                            )
                return pcs

            def out_block(b, ot, r0, pcs, engine="scalar"):
                n = len(pcs)
                oh = out_pool.tile([P, 8 * n, W], F32, name=f"oh{b}{ot}{r0}", tag="oh")
                for c in range(n):
                    if engine == "scalar":
                        nc.scalar.activation(
                            oh[:, c * 8:(c + 1) * 8, :], pcs[c][:], AF.Identity,
                            bias=bias_col[:, ot:ot + 1], scale=winv[ot][:, b:b + 1],
                        )
                    else:
                        nc.vector.tensor_scalar(
                            oh[:, c * 8:(c + 1) * 8, :], pcs[c][:],
                            winv[ot][:, b:b + 1], bias_col[:, ot:ot + 1],
                            mybir.AluOpType.mult, mybir.AluOpType.add,
                        )
                nc.sync.dma_start(
                    out.ap()[b, ot * P:(ot + 1) * P, r0:r0 + 8 * n, :], oh[:])

            def _mid_t01():
                cast_w(0, 1, nc.vector)
                transpose_wblock(0, 1)

            if MID_T01:
                pcs_h0 = mm_block(0, 0, 0, 4, mid_cb=_mid_t01)
            else:
                _mid_t01()
                pcs_h0 = mm_block(0, 0, 0, 4)

            # ---------- demod path, emitted so the in-order PE never stalls -------
            # w^2 + kk-reduce (DVE), W2T via 4 PE transposes, sigma matmul, rsqrt
            w2_nat = []
            for ot in range(NO):
                sq = temps.tile([P, CIN, KK], F32, name=f"sq{ot}", tag="sq", bufs=1)
                sq_i = nc.vector.tensor_mul(sq[:], w_nat[ot][:], w_nat[ot][:])
                # keep the w^2 work behind the critical sample-0 casts
                bass._add_dep_helper(sq_i.ins, last_cast_b0.ins, sync=False,
                                     reason="w2 after b0 x casts")
                t = consts.tile([P, CIN], F32, name=f"w2n{ot}", tag=f"w2n{ot}")
                nc.vector.reduce_sum(t[:], sq[:], axis=mybir.AxisListType.X)
                w2_nat.append(t)

            cast_w(1, 0, nc.vector)
            transpose_wblock(1, 0)
            cast_w(1, 1, nc.vector)
            transpose_wblock(1, 1)

            identity_f = consts.tile([P, P], F32)
            make_identity(nc, identity_f)
            w2t = [consts.tile([P, COUT], F32, name=f"w2t{i}", tag=f"w2t{i}")
                   for i in range(NI)]
            for it in range(NI):
                pt = psum.tile([P, NO, P], F32, name=f"w2tp{it}",
                               tag=f"ch{tp_idx[0] % 4}")
                tp_idx[0] += 1
                for ot in range(NO):
                    nc.tensor.transpose(pt[:, ot, :],
                                        w2_nat[ot][:, it * P:(it + 1) * P],
                                        identity_f[:])
                nc.scalar.copy(w2t[it][:], pt[:].rearrange("p o i -> p (o i)"))

            winv = []
            for ot in range(NO):
                ps = psum.tile([P, B_LOC], F32, name=f"sig{ot}", tag=f"ch{ot}")
                for it in range(NI):
                    nc.tensor.matmul(
                        ps[:], w2t[it][:, ot * P:(ot + 1) * P], style2[it][:],
                        start=(it == 0), stop=(it == NI - 1),
                    )
                wstd = consts.tile([P, B_LOC], F32, name=f"wstd{ot}", tag=f"wstd{ot}")
                nc.scalar.activation(wstd[:], ps[:], AF.Sqrt, bias=eps_col[:])
                wi = consts.tile([P, B_LOC], F32, name=f"winv{ot}", tag=f"winv{ot}")
                nc.vector.reciprocal(wi[:], wstd[:])
                winv.append(wi)

            # ---------- rest of the schedule ----------
            out_block(0, 0, 0, pcs_h0)
            out_block(0, 0, 32, mm_block(0, 0, 32, 4))
            for half in range(2):
                out_block(0, 1, half * 32, mm_block(0, 1, half * 32, 4))

            # sample 1 input stage
            xs_borders(1)
            for it in range(NI):
                for part in range(2):
                    xs_cast(1, it, part)

            for half in range(2):
                out_block(1, 0, half * 32, mm_block(1, 0, half * 32, 4))
            out_block(1, 1, 0, mm_block(1, 1, 0, 4))
            # final blocks shrink progressively so the drain tail is short;
            # optionally the last two drain on DVE so ACT and DVE overlap
            tail_eng = "vector" if VEC_OUT else "scalar"
            if FINE_TAIL:
                out_block(1, 1, 32, mm_block(1, 1, 32, 2, ctag0=0))
                out_block(1, 1, 48, mm_block(1, 1, 48, 1, ctag0=2), engine=tail_eng)
                out_block(1, 1, 56, mm_block(1, 1, 56, 1, ctag0=3), engine=tail_eng)
            else:
                out_block(1, 1, 32, mm_block(1, 1, 32, 4), engine=tail_eng)
    return nc


_CACHED_NC = None


def _get_nc():
    global _CACHED_NC
    if _CACHED_NC is None:
        nc = bacc.Bacc("TRN2", target_bir_lowering=False, debug=False,
                       num_devices=N_CORES)
        build_conv2dmod(nc)
        nc.compile()
        _CACHED_NC = nc
    return _CACHED_NC


def kernel(x, y, weights, bias, style_w, style_b, _trace=False):
    x = np.ascontiguousarray(np.asarray(x, dtype=np.float32))
    y = np.ascontiguousarray(np.asarray(y, dtype=np.float32))
    weights = np.ascontiguousarray(np.asarray(weights, dtype=np.float32))
    bias = np.ascontiguousarray(np.asarray(bias, dtype=np.float32))
    style_w = np.ascontiguousarray(np.asarray(style_w, dtype=np.float32))
    style_b = np.ascontiguousarray(np.asarray(style_b, dtype=np.float32))

    nc = _get_nc()
    in_maps = [
        {
            "x": np.ascontiguousarray(x[c * B_LOC:(c + 1) * B_LOC]),
            "y": np.ascontiguousarray(y[c * B_LOC:(c + 1) * B_LOC]),
            "weights": weights,
            "bias": bias,
            "style_w": style_w,
            "style_b": style_b,
        }
        for c in range(N_CORES)
    ]
    res = bass_utils.run_bass_kernel_spmd(
        nc, in_maps, core_ids=list(range(N_CORES)), trace=_trace
    )
    out = np.concatenate([r["out"] for r in res.results], axis=0)
    if _trace:
        kernel.last_results = res
    return out



# revision 3
# speedup vs baseline: 1.0304x; 1.0304x over previous
"""Modulated Conv2D (StyleGAN2-style) Trainium2 Bass kernel.

Problem shapes (hardcoded):
  x: [16, 256, 64, 64] f32    y: [16, 512] f32
  weights: [256, 256, 3, 3]   bias: [256]
  style_w: [256, 512]         style_b: [256]
  out: [16, 256, 64, 64] f32

Formulation: fold the per-sample style modulation into the weights
(classic StyleGAN2), so x needs no per-pixel scaling at all:
  w_mod[b][i,kk,o] = wT[i,kk,o] * style[b,i]
  out[b,o] = (1/wstd[b,o]) * conv(x[b], w_mod[b])[o] + bias[o]
  wstd[b,o] = sqrt(sum_{i,kk} wT[i,kk,o]^2 * style[b,i]^2 + eps)

Host-side layout prep (pure data movement / dtype packing):
  - wT = weights transposed to [Cin, kk, Cout] and cast bf16, so the PE
    needs no transposes and lhsT slices are contiguous.
  - x zero-padded to [66,66] and cast bf16, so DMA lands matmul-ready
    tiles directly (no on-device pad/scale pass, half the bytes).
  - device output is bf16 (upcast to f32 on host); rel-err budget 2e-2,
    bf16 out costs ~2e-3.

Sharding: data-parallel over batch, 2 samples per core across 8 cores.
Conv computed as 9 shifted matmuls per Cin-block (2) accumulating in
PSUM f32, 8-row x 64-col chunks (512-elem free dim = 1 PSUM bank).

Engines: PE does conv matmuls + the tiny sigma matmul; DVE does style,
per-sample weight modulation, W2 and issues output DMAs; Scalar drains
PSUM (scale by 1/wstd + bias, f32->bf16) and loads the wT DMAs; GPSIMD
ring loads x; Sync ring loads the style path.
"""

import numpy as np
import ml_dtypes

import concourse.bass as bass
import concourse.tile as tile
from concourse import bacc, mybir
from concourse import bass_utils

EPS = 1e-8
P = 128
B_LOC = 2          # samples per core
B_FULL = 16
CIN, COUT = 256, 256
NI, NO = CIN // P, COUT // P   # 2, 2
S = 512
H = W = 64
KK = 9             # 3x3 taps
HP, WP = H + 2, W + 2  # zero-padded image
N_CORES = 8
ROWS_A = 34        # rows in the first half of each x tile load

F32 = mybir.dt.float32
BF16 = mybir.dt.bfloat16
AF = mybir.ActivationFunctionType


def build_conv2dmod(nc):
    xp = nc.dram_tensor("xp", [B_LOC, CIN, HP, WP], BF16, kind="ExternalInput")
    y = nc.dram_tensor("y", [B_LOC, S], F32, kind="ExternalInput")
    wt = nc.dram_tensor("wt", [CIN, KK, COUT], BF16, kind="ExternalInput")
    bias = nc.dram_tensor("bias", [COUT], F32, kind="ExternalInput")
    style_w = nc.dram_tensor("style_w", [CIN, S], F32, kind="ExternalInput")
    style_b = nc.dram_tensor("style_b", [CIN], F32, kind="ExternalInput")
    out = nc.dram_tensor("out", [B_LOC, COUT, H, W], BF16, kind="ExternalOutput")

    with tile.TileContext(nc) as tc:
        with (
            tc.tile_pool(name="consts", bufs=1) as consts,
            tc.tile_pool(name="temps", bufs=1) as temps,
            tc.tile_pool(name="wmod_pool", bufs=2) as wmod_pool,
            tc.tile_pool(name="xs_pool", bufs=1) as xs_pool,
            tc.tile_pool(name="out_pool", bufs=3) as out_pool,
            tc.tile_pool(name="psum", bufs=2, space="PSUM") as psum,
        ):
            # ---------------- tiles ----------------
            sw_nat = [consts.tile([P, S], F32, name=f"sw{i}", tag=f"sw{i}")
                      for i in range(NI)]
            y_bcast = consts.tile([P, B_LOC, S], F32)
            wt_t = [consts.tile([P, KK, COUT], BF16, name=f"wt{i}", tag=f"wt{i}")
                    for i in range(NI)]
            xs = {}
            for b in range(B_LOC):
                for it in range(NI):
                    xs[(b, it)] = xs_pool.tile([P, HP, WP], BF16,
                                               name=f"xs{b}_{it}", tag=f"xs{b}_{it}")
            bias_col = consts.tile([P, NO], F32)
            style_b_col = consts.tile([P, NI], F32)

            # ------------- DMA rings, ordered by when they gate compute -------
            # sync ring: style path first (gates everything via style_col)
            nc.sync.dma_start(y_bcast[:], y.ap()[None].to_broadcast((P, B_LOC, S)))
            nc.sync.dma_start(sw_nat[0][:], style_w.ap()[0:P, :])
            nc.sync.dma_start(sw_nat[1][:], style_w.ap()[P:2 * P, :])
            nc.sync.dma_start(bias_col[:], bias.ap().rearrange("(oo oi) -> oi oo", oi=P))
            nc.sync.dma_start(style_b_col[:],
                              style_b.ap().rearrange("(io ii) -> ii io", ii=P))
            # scalar ring: the transposed weights
            nc.scalar.dma_start(wt_t[0][:], wt.ap()[0:P])
            nc.scalar.dma_start(wt_t[1][:], wt.ap()[P:2 * P])

            # gpsimd ring: padded x, first-needed halves first
            def load_x(b, it, half):
                r = slice(0, ROWS_A) if half == 0 else slice(ROWS_A, HP)
                nc.gpsimd.dma_start(xs[(b, it)][:, r, :],
                                    xp.ap()[b, it * P:(it + 1) * P, r, :])

            for b in range(B_LOC):
                for half in range(2):
                    for it in range(NI):
                        load_x(b, it, half)

            # pre-warm the ACT function table that Sqrt lives in
            eps_col = consts.tile([P, 1], F32)
            nc.gpsimd.memset(eps_col[:], EPS)
            lafs_warm = consts.tile([P, 1], F32)
            nc.scalar.activation(lafs_warm[:], eps_col[:], AF.Sqrt)

            # ---------- style (DVE): [P(cin), B_LOC] per cin block ----------
            style_col = []
            style2 = []
            for it in range(NI):
                sc = consts.tile([P, B_LOC], F32, name=f"stc{it}", tag=f"stc{it}")
                s2 = consts.tile([P, B_LOC], F32, name=f"st2{it}", tag=f"st2{it}")
                tmp = temps.tile([P, B_LOC, S], F32, name=f"stmp{it}", tag="stmp")
                nc.vector.tensor_mul(
                    tmp[:], y_bcast[:],
                    sw_nat[it][:, None, :].to_broadcast((P, B_LOC, S)))
                nc.vector.reduce_sum(sc[:], tmp[:], axis=mybir.AxisListType.X)
                nc.vector.tensor_scalar_add(sc[:], sc[:], style_b_col[:, it:it + 1])
                nc.vector.tensor_mul(s2[:], sc[:], sc[:])
                style_col.append(sc)
                style2.append(s2)

            # ---------- per-sample modulated weights (DVE) ----------
            w_mod = {}

            def make_wmod(b, it):
                t = wmod_pool.tile([P, KK, COUT], BF16, name=f"wm{b}_{it}",
                                   tag=f"wm{it}")
                nc.vector.tensor_scalar_mul(t[:], wt_t[it][:],
                                            style_col[it][:, b:b + 1])
                w_mod[(b, it)] = t

            make_wmod(0, 0)
            make_wmod(0, 1)

            # ---------- main conv block: 18*nchunks matmuls per call ----------
            def mm_block(b, ot, r0, nchunks, ctag0=0):
                pcs = [psum.tile([P, 8, W], F32, name=f"pc{b}{ot}{r0}_{c}",
                                 tag=f"ch{(ctag0 + c) % 4}")
                       for c in range(nchunks)]
                first, last = (0, 0), (NI - 1, KK - 1)
                for it in range(NI):
                    for kk in range(KK):
                        dy, dx = kk // 3, kk % 3
                        lhsT = w_mod[(b, it)][:, kk, ot * P:(ot + 1) * P]
                        for c in range(nchunks):
                            rs = r0 + c * 8 + dy
                            nc.tensor.matmul(
                                pcs[c][:], lhsT, xs[(b, it)][:, rs:rs + 8, dx:dx + W],
                                start=((it, kk) == first), stop=((it, kk) == last),
                            )
                return pcs

            def out_block(b, ot, r0, pcs, dma_rows=16, engine="scalar"):
                n = len(pcs)
                oh = out_pool.tile([P, 8 * n, W], BF16, name=f"oh{b}{ot}{r0}",
                                   tag="oh")
                done = 0
                for c in range(n):
                    if engine == "scalar":
                        nc.scalar.activation(
                            oh[:, c * 8:(c + 1) * 8, :], pcs[c][:], AF.Identity,
                            bias=bias_col[:, ot:ot + 1], scale=winv[ot][:, b:b + 1],
                        )
                    else:
                        nc.vector.tensor_scalar(
                            oh[:, c * 8:(c + 1) * 8, :], pcs[c][:],
                            winv[ot][:, b:b + 1], bias_col[:, ot:ot + 1],
                            mybir.AluOpType.mult, mybir.AluOpType.add,
                        )
                    rows = (c + 1) * 8
                    if rows - done >= dma_rows or c == n - 1:
                        nc.sync.dma_start(
                            out.ap()[b, ot * P:(ot + 1) * P, r0 + done:r0 + rows, :],
                            oh[:, done:rows, :])
                        done = rows

            # first conv block for sample 0 goes as early as possible
            pcs_h0 = mm_block(0, 0, 0, 4)

            # ---------- demod path (emitted behind the first block) ----------
            # W2T[i_part, o] = sum_kk wT[i,kk,o]^2  (DVE, f32)
            w2t = [consts.tile([P, COUT], F32, name=f"w2t{i}", tag=f"w2t{i}")
                   for i in range(NI)]
            sq = temps.tile([P, KK, COUT], F32, name="sq", tag="sq")
            for it in range(NI):
                nc.vector.tensor_mul(sq[:], wt_t[it][:], wt_t[it][:])
                nc.vector.reduce_sum(w2t[it][:],
                                     sq[:].rearrange("p kk o -> p o kk"),
                                     axis=mybir.AxisListType.X)

            # sigma[o_part, b] = sum_i W2T[i,o] * style2[i,b]  (PE, f32)
            winv = []
            for ot in range(NO):
                ps = psum.tile([P, B_LOC], F32, name=f"sig{ot}", tag=f"ch{ot}")
                for it in range(NI):
                    nc.tensor.matmul(
                        ps[:], w2t[it][:, ot * P:(ot + 1) * P], style2[it][:],
                        start=(it == 0), stop=(it == NI - 1),
                    )
                wstd = consts.tile([P, B_LOC], F32, name=f"wstd{ot}", tag=f"wstd{ot}")
                nc.scalar.activation(wstd[:], ps[:], AF.Sqrt, bias=eps_col[:])
                wi = consts.tile([P, B_LOC], F32, name=f"winv{ot}", tag=f"winv{ot}")
                nc.vector.reciprocal(wi[:], wstd[:])
                winv.append(wi)

            # sample-1 modulated weights, before DVE starts blocking on
            # out-DMA waits
            make_wmod(1, 0)
            make_wmod(1, 1)

            # ---------- rest of the schedule ----------
            out_block(0, 0, 0, pcs_h0)
            out_block(0, 0, 32, mm_block(0, 0, 32, 4))
            for half in range(2):
                out_block(0, 1, half * 32, mm_block(0, 1, half * 32, 4))
            for half in range(2):
                out_block(1, 0, half * 32, mm_block(1, 0, half * 32, 4))
            out_block(1, 1, 0, mm_block(1, 1, 0, 4))
            # shrinking tail so the final drain after the last matmul is short
            out_block(1, 1, 32, mm_block(1, 1, 32, 2, ctag0=0), dma_rows=8)
            out_block(1, 1, 48, mm_block(1, 1, 48, 1, ctag0=2), dma_rows=8)
            out_block(1, 1, 56, mm_block(1, 1, 56, 1, ctag0=3), dma_rows=8)
    return nc


_CACHED_NC = None


def _get_nc():
    global _CACHED_NC
    if _CACHED_NC is None:
        nc = bacc.Bacc("TRN2", target_bir_lowering=False, debug=False,
                       num_devices=N_CORES)
        build_conv2dmod(nc)
        nc.compile()
        _CACHED_NC = nc
    return _CACHED_NC


def kernel(x, y, weights, bias, style_w, style_b, _trace=False):
    x = np.asarray(x, dtype=np.float32)
    y = np.ascontiguousarray(np.asarray(y, dtype=np.float32))
    weights = np.asarray(weights, dtype=np.float32)
    bias = np.ascontiguousarray(np.asarray(bias, dtype=np.float32))
    style_w = np.ascontiguousarray(np.asarray(style_w, dtype=np.float32))
    style_b = np.ascontiguousarray(np.asarray(style_b, dtype=np.float32))

    # host-side layout packing: transpose weights to [Cin, kk, Cout] bf16,
    # zero-pad x to [66, 66] bf16
    wt = np.ascontiguousarray(
        weights.transpose(1, 2, 3, 0).reshape(CIN, KK, COUT)
    ).astype(ml_dtypes.bfloat16)
    xp = np.zeros((B_FULL, CIN, HP, WP), dtype=ml_dtypes.bfloat16)
    xp[:, :, 1:H + 1, 1:W + 1] = x.astype(ml_dtypes.bfloat16)

    nc = _get_nc()
    in_maps = [
        {
            "xp": np.ascontiguousarray(xp[c * B_LOC:(c + 1) * B_LOC]),
            "y": np.ascontiguousarray(y[c * B_LOC:(c + 1) * B_LOC]),
            "wt": wt,
            "bias": bias,
            "style_w": style_w,
            "style_b": style_b,
        }
        for c in range(N_CORES)
    ]
    res = bass_utils.run_bass_kernel_spmd(
        nc, in_maps, core_ids=list(range(N_CORES)), trace=_trace
    )
    out = np.concatenate([r["out"] for r in res.results], axis=0).astype(np.float32)
    if _trace:
        kernel.last_results = res
    return out


# revision 5
# speedup vs baseline: 1.1294x; 1.0961x over previous
"""Modulated Conv2D (StyleGAN2-style) Trainium2 Bass kernel.

Problem shapes (hardcoded):
  x: [16, 256, 64, 64] f32    y: [16, 512] f32
  weights: [256, 256, 3, 3]   bias: [256]
  style_w: [256, 512]         style_b: [256]
  out: [16, 256, 64, 64] f32

Formulation: fold the per-sample style modulation into the weights
(classic StyleGAN2), so x needs no per-pixel scaling at all:
  style[b,i] = y[b] @ style_w[i] + style_b[i]      (tiny PE matmul)
  w_mod[b][i,kk,o] = wT[i,kk,o] * style[b,i]       (DVE)
  out[b,o] = (1/wstd[b,o]) * conv(x[b], w_mod[b])[o] + bias[o]
  wstd[b,o] = sqrt(sum_{i,kk} wT[i,kk,o]^2 * style[b,i]^2 + eps)

Host-side layout prep (pure data movement / dtype packing):
  - wT = weights transposed to [Cin, kk, Cout], bf16: no PE transposes,
    contiguous lhsT slices.
  - swT/yT = style_w.T / y.T in bf16 so style is a [512]-contraction
    PE matmul (avoids a slow broadcast DMA of y).
  - x zero-padded to [66,66], bf16: DMA lands matmul-ready tiles (no
    on-device pad/scale pass, half the bytes).
  - device output bf16, upcast on host (budget 2e-2, bf16 out ~2e-3).

Sharding: data-parallel over batch, 2 samples per core across 8 cores.
Conv = 9 shifted matmuls per Cin-block (x2) accumulating in PSUM f32,
8-row x 64-col chunks (512-elem free dim = one PSUM bank).

Engines: PE does style + sigma + conv matmuls; DVE squares style and
modulates the per-sample weights; GPSIMD computes W2 = sum_kk wT^2;
Scalar drains PSUM (scale 1/wstd + bias, f32->bf16); the two HW DMA
rings split: scalar ring loads weights, sync ring loads x + stores out.
"""

import numpy as np
import ml_dtypes

import concourse.bass as bass
import concourse.tile as tile
from concourse import bacc, mybir
from concourse import bass_utils

EPS = 1e-8
P = 128
B_LOC = 2          # samples per core
B_FULL = 16
CIN, COUT = 256, 256
NI, NO = CIN // P, COUT // P   # 2, 2
S = 512
NS = S // P        # 4 style contraction blocks
KK = 9             # 3x3 taps
H = W = 64
HP, WP = H + 2, W + 2  # zero-padded image
N_CORES = 8
ROWS_A = 34        # rows in the first half of each x tile load

F32 = mybir.dt.float32
BF16 = mybir.dt.bfloat16
AF = mybir.ActivationFunctionType


def _chain(instrs, reason):
    """Force program order on one engine (guides the tile scheduler)."""
    for a, b in zip(instrs[1:], instrs[:-1]):
        bass._add_dep_helper(a.ins, b.ins, sync=False, reason=reason)


def build_conv2dmod(nc):
    xp = nc.dram_tensor("xp", [B_LOC, CIN, HP, WP], BF16, kind="ExternalInput")
    yt = nc.dram_tensor("yt", [S, B_LOC], BF16, kind="ExternalInput")
    wt = nc.dram_tensor("wt", [CIN, KK, COUT], BF16, kind="ExternalInput")
    swt = nc.dram_tensor("swt", [S, CIN], BF16, kind="ExternalInput")
    bias = nc.dram_tensor("bias", [COUT], F32, kind="ExternalInput")
    style_b = nc.dram_tensor("style_b", [CIN], F32, kind="ExternalInput")
    out = nc.dram_tensor("out", [B_LOC, COUT, H, W], BF16, kind="ExternalOutput")

    with tile.TileContext(nc) as tc:
        with (
            tc.tile_pool(name="consts", bufs=1) as consts,
            tc.tile_pool(name="temps", bufs=1) as temps,
            tc.tile_pool(name="wmod_pool", bufs=2) as wmod_pool,
            tc.tile_pool(name="xs_pool", bufs=1) as xs_pool,
            tc.tile_pool(name="out_pool", bufs=3) as out_pool,
            tc.tile_pool(name="psum", bufs=2, space="PSUM") as psum,
        ):
            # ---------------- tiles ----------------
            yt_t = consts.tile([P, NS, B_LOC], BF16)
            swt_t = consts.tile([P, NS, CIN], BF16)
            wt_t = [consts.tile([P, KK, COUT], BF16, name=f"wt{i}", tag=f"wt{i}")
                    for i in range(NI)]
            xs = {}
            for b in range(B_LOC):
                for it in range(NI):
                    xs[(b, it)] = xs_pool.tile([P, HP, WP], BF16,
                                               name=f"xs{b}_{it}", tag=f"xs{b}_{it}")
            bias_col = consts.tile([P, NO], F32)
            style_b_col = consts.tile([P, NI], F32)

            # ------------- DMA rings, ordered by when they gate compute -------
            # scalar HW ring: style operands then the transposed weights
            nc.scalar.dma_start(yt_t[:], yt.ap().rearrange("(sb sp) b -> sp sb b", sp=P))
            nc.scalar.dma_start(swt_t[:], swt.ap().rearrange("(sb sp) i -> sp sb i", sp=P))
            nc.scalar.dma_start(wt_t[0][:], wt.ap()[0:P])
            nc.scalar.dma_start(wt_t[1][:], wt.ap()[P:2 * P])

            # sync HW ring: small consts, then padded x halves in need order
            nc.sync.dma_start(bias_col[:], bias.ap().rearrange("(oo oi) -> oi oo", oi=P))
            nc.sync.dma_start(style_b_col[:],
                              style_b.ap().rearrange("(io ii) -> ii io", ii=P))

            def load_x(b, it, half):
                r = slice(0, ROWS_A) if half == 0 else slice(ROWS_A, HP)
                nc.sync.dma_start(xs[(b, it)][:, r, :],
                                  xp.ap()[b, it * P:(it + 1) * P, r, :])

            for b in range(B_LOC):
                for half in range(2):
                    for it in range(NI):
                        load_x(b, it, half)

            # pre-warm the ACT function table that Sqrt lives in
            eps_col = consts.tile([P, 1], F32)
            nc.gpsimd.memset(eps_col[:], EPS)
            lafs_warm = consts.tile([P, 1], F32)
            nc.scalar.activation(lafs_warm[:], eps_col[:], AF.Sqrt)

            # ---------- style (PE): [P(cin), B_LOC] per cin block ----------
            style_col = []
            style2 = []
            style_sq_i = []
            for it in range(NI):
                ps = psum.tile([P, B_LOC], F32, name=f"styp{it}", tag=f"ch{2 + it}")
                for sb in range(NS):
                    nc.tensor.matmul(
                        ps[:], swt_t[:, sb, it * P:(it + 1) * P], yt_t[:, sb, :],
                        start=(sb == 0), stop=(sb == NS - 1),
                    )
                sc = consts.tile([P, B_LOC], F32, name=f"stc{it}", tag=f"stc{it}")
                nc.scalar.activation(sc[:], ps[:], AF.Identity,
                                     bias=style_b_col[:, it:it + 1])
                s2 = consts.tile([P, B_LOC], F32, name=f"st2{it}", tag=f"st2{it}")
                style_sq_i.append(nc.vector.tensor_mul(s2[:], sc[:], sc[:]))
                style_col.append(sc)
                style2.append(s2)

            # ---------- per-sample modulated weights (DVE) ----------
            w_mod = {}
            wmod_i = []

            def make_wmod(b, it):
                t = wmod_pool.tile([P, KK, COUT], BF16, name=f"wm{b}_{it}",
                                   tag=f"wm{it}")
                wmod_i.append(nc.vector.tensor_scalar_mul(
                    t[:], wt_t[it][:], style_col[it][:, b:b + 1]))
                w_mod[(b, it)] = t

            make_wmod(0, 0)
            make_wmod(0, 1)
            make_wmod(1, 0)
            make_wmod(1, 1)
            _chain(style_sq_i + wmod_i, "style squares then wmods in order")

            # ---------- main conv block: 18*nchunks matmuls per call ----------
            def mm_block(b, ot, r0, nchunks, ctag0=0):
                pcs = [psum.tile([P, 8, W], F32, name=f"pc{b}{ot}{r0}_{c}",
                                 tag=f"ch{(ctag0 + c) % 4}")
                       for c in range(nchunks)]
                first, last = (0, 0), (NI - 1, KK - 1)
                for it in range(NI):
                    for kk in range(KK):
                        dy, dx = kk // 3, kk % 3
                        lhsT = w_mod[(b, it)][:, kk, ot * P:(ot + 1) * P]
                        for c in range(nchunks):
                            rs = r0 + c * 8 + dy
                            nc.tensor.matmul(
                                pcs[c][:], lhsT, xs[(b, it)][:, rs:rs + 8, dx:dx + W],
                                start=((it, kk) == first), stop=((it, kk) == last),
                            )
                return pcs

            def out_block(b, ot, r0, pcs, dma_rows=16, engine="scalar"):
                n = len(pcs)
                oh = out_pool.tile([P, 8 * n, W], BF16, name=f"oh{b}{ot}{r0}",
                                   tag="oh")
                done = 0
                for c in range(n):
                    if engine == "scalar":
                        nc.scalar.activation(
                            oh[:, c * 8:(c + 1) * 8, :], pcs[c][:], AF.Identity,
                            bias=bias_col[:, ot:ot + 1], scale=winv[ot][:, b:b + 1],
                        )
                    else:
                        nc.vector.tensor_scalar(
                            oh[:, c * 8:(c + 1) * 8, :], pcs[c][:],
                            winv[ot][:, b:b + 1], bias_col[:, ot:ot + 1],
                            mybir.AluOpType.mult, mybir.AluOpType.add,
                        )
                    rows = (c + 1) * 8
                    if rows - done >= dma_rows or c == n - 1:
                        nc.sync.dma_start(
                            out.ap()[b, ot * P:(ot + 1) * P, r0 + done:r0 + rows, :],
                            oh[:, done:rows, :])
                        done = rows

            # first conv block for sample 0 goes as early as possible
            pcs_h0 = mm_block(0, 0, 0, 4)

            # ---------- demod path (behind the first block) ----------
            # W2T[i_part, o] = sum_kk wT[i,kk,o]^2; GPSIMD squares into a
            # [p, o, kk] layout so the DVE reduce reads contiguously
            w2t = [consts.tile([P, COUT], F32, name=f"w2t{i}", tag=f"w2t{i}")
                   for i in range(NI)]
            sqs = [temps.tile([P, COUT, KK], F32, name=f"sq{i}", tag=f"sq{i}")
                   for i in range(NI)]
            red_i = []
            for it in range(NI):
                nc.gpsimd.tensor_mul(sqs[it][:].rearrange("p o kk -> p kk o"),
                                     wt_t[it][:], wt_t[it][:])
                red_i.append(nc.vector.reduce_sum(w2t[it][:], sqs[it][:],
                                                  axis=mybir.AxisListType.X))
            _chain([wmod_i[-1]] + red_i, "w2 reduces after wmods")

            # sigma[o_part, b] = sum_i W2T[i,o] * style2[i,b]  (PE, f32)
            winv = []
            for ot in range(NO):
                ps = psum.tile([P, B_LOC], F32, name=f"sig{ot}", tag=f"ch{ot}")
                for it in range(NI):
                    nc.tensor.matmul(
                        ps[:], w2t[it][:, ot * P:(ot + 1) * P], style2[it][:],
                        start=(it == 0), stop=(it == NI - 1),
                    )
                wstd = consts.tile([P, B_LOC], F32, name=f"wstd{ot}", tag=f"wstd{ot}")
                nc.scalar.activation(wstd[:], ps[:], AF.Sqrt, bias=eps_col[:])
                wi = consts.tile([P, B_LOC], F32, name=f"winv{ot}", tag=f"winv{ot}")
                nc.vector.reciprocal(wi[:], wstd[:])
                winv.append(wi)

            # ---------- rest of the schedule ----------
            out_block(0, 0, 0, pcs_h0)
            out_block(0, 0, 32, mm_block(0, 0, 32, 4))
            for half in range(2):
                out_block(0, 1, half * 32, mm_block(0, 1, half * 32, 4))
            for half in range(2):
                out_block(1, 0, half * 32, mm_block(1, 0, half * 32, 4))
            out_block(1, 1, 0, mm_block(1, 1, 0, 4))
            # shrinking tail so the final drain after the last matmul is short
            out_block(1, 1, 32, mm_block(1, 1, 32, 2, ctag0=0), dma_rows=8)
            out_block(1, 1, 48, mm_block(1, 1, 48, 1, ctag0=2), dma_rows=8)
            out_block(1, 1, 56, mm_block(1, 1, 56, 1, ctag0=3), dma_rows=8)
    return nc


_CACHED_NC = None


def _get_nc():
    global _CACHED_NC
    if _CACHED_NC is None:
        nc = bacc.Bacc("TRN2", target_bir_lowering=False, debug=False,
                       num_devices=N_CORES)
        build_conv2dmod(nc)
        nc.compile()
        _CACHED_NC = nc
    return _CACHED_NC


def kernel(x, y, weights, bias, style_w, style_b, _trace=False):
    x = np.asarray(x, dtype=np.float32)
    y = np.asarray(y, dtype=np.float32)
    weights = np.asarray(weights, dtype=np.float32)
    bias = np.ascontiguousarray(np.asarray(bias, dtype=np.float32))
    style_w = np.asarray(style_w, dtype=np.float32)
    style_b = np.ascontiguousarray(np.asarray(style_b, dtype=np.float32))

    # host-side layout packing: transpose weights to [Cin, kk, Cout] bf16,
    # style matmul operands transposed bf16, x zero-padded [66, 66] bf16
    wt = np.ascontiguousarray(
        weights.transpose(1, 2, 3, 0).reshape(CIN, KK, COUT)
    ).astype(ml_dtypes.bfloat16)
    swt = np.ascontiguousarray(style_w.T).astype(ml_dtypes.bfloat16)
    ytf = np.ascontiguousarray(y.T).astype(ml_dtypes.bfloat16)  # [S, B_FULL]
    xp = np.zeros((B_FULL, CIN, HP, WP), dtype=ml_dtypes.bfloat16)
    xp[:, :, 1:H + 1, 1:W + 1] = x.astype(ml_dtypes.bfloat16)

    nc = _get_nc()
    in_maps = [
        {
            "xp": np.ascontiguousarray(xp[c * B_LOC:(c + 1) * B_LOC]),
            "yt": np.ascontiguousarray(ytf[:, c * B_LOC:(c + 1) * B_LOC]),
            "wt": wt,
            "swt": swt,
            "bias": bias,
            "style_b": style_b,
        }
        for c in range(N_CORES)
    ]
    res = bass_utils.run_bass_kernel_spmd(
        nc, in_maps, core_ids=list(range(N_CORES)), trace=_trace
    )
    out = np.concatenate([r["out"] for r in res.results], axis=0).astype(np.float32)
    if _trace:
        kernel.last_results = res
    return out
